# revision 21
# baseline (speedup 1.0000x reference)
"""Trainium2 Bass kernel for nn_DiscriminatorForMissing (NaN branch).

Data-parallel over batch: each of 8 cores gets B/8 = 1024 rows, with the
per-distribution log-likelihoods q[k] AllReduced across cores before the
softmax mixing.

Math notes (m = isnan(x), xm = where(m, 0, x)):
  q[k] reduces exactly to masked per-column batch stats (cnt, Sx, Sxx):
    q[k] = sum_d cnt[d]*(ln(g+c_kd) + means_kd^2/(g+c_kd) + LOG_2PI)
         + sum_d Sxx[d]/(g+c_kd) - 2*sum_d Sx[d]*means_kd/(g+c_kd)
  r = softmax(-q/2 + log_p) is numerically one-hot for any realistic
  input (|q| gaps ~1e5 >> fp32 softmax range), so the K-mixture collapses
  to the argmax distribution: we select means[k*]/covs[k*] with a matmul
  against the one-hot mask (r == max(r)) and scale by the true r_k*.

  relu-moment: nr(mu, sig) = s*phi(w) + mu*Phi(w), w = mu/s, s = sqrt(sig)
    = exp(0.5*(ln sig - w^2) + ln(c1*r)) + exp(0.5*ln sig + ln r)*Gelu(w)
  -> only {Ln, Exp} + {Gelu} ACT table sets; phases fenced to avoid
  table-set thrashing.

All heavy matmuls run as float32r (1 cyc/row, ~13-bit mantissa).
"""

import os
import sys
import tempfile

import numpy as np
import ml_dtypes

if "/opt/trn_rl_repo" not in sys.path:
    sys.path.insert(0, "/opt/trn_rl_repo")

import concourse.bass as bass  # noqa: E402,F401
import concourse.tile as tile  # noqa: E402
from concourse import mybir, bacc  # noqa: E402
from concourse.bass_utils import run_bass_kernel_spmd  # noqa: E402
from bass_rust import add_dep_helper  # noqa: E402

B, D, H0, H1, K = 8192, 512, 1024, 1024, 10
N_CORES = 8
BS = B // N_CORES          # rows per core
BT = 512                   # batch tile (free dim)
NBT = BS // BT             # 2 batch tiles
DC = D // 128              # 4 contraction chunks
HC = H0 // 128             # 8 hidden chunks
H1C = H1 // 128            # 8 layer-2 chunks
KP = 16                    # padded K for [1,16] vectors

LOG_2PI = float(np.log(2.0 * np.pi))
INV_SQRT_2PI = 0.3989422804014327
NEG_BIG = -1.0e30

F32 = mybir.dt.float32
F32R = mybir.dt.float32r
BF16 = mybir.dt.bfloat16
AF = mybir.ActivationFunctionType
ALU = mybir.AluOpType
AX = mybir.AxisListType

_CACHE = {}
FENCES = [True]
PBCAST = [True]


def _setup_act_tables():
    """Point walrus at a reordered act_info.json so Ln and Exp both resolve
    to natural_log_exp_and_others (one ACT table set instead of two, which
    otherwise costs a ~1.3us ACT_TABLE_LOAD at every Ln<->Exp boundary)."""
    if "act_json" in _CACHE:
        os.environ["BASS_ACT_ROOT_JSON_PATH"] = _CACHE["act_json"]
        return
    import json
    import neuronxcc
    src_dir = os.path.join(os.path.dirname(neuronxcc.__file__),
                           "pwp", "pwp_bin_trainium")
    dst_dir = os.path.join(tempfile.mkdtemp(prefix="act_pwp_"), "pwp")
    os.makedirs(dst_dir, exist_ok=True)
    for f in os.listdir(src_dir):
        if f != "act_info.json":
            os.symlink(os.path.join(src_dir, f), os.path.join(dst_dir, f))
    d = json.load(open(os.path.join(src_dir, "act_info.json")))
    sets = d["act_func_sets"]
    sets.sort(key=lambda s: 0 if s["name"] == "natural_log_exp_and_others" else 1)
    path = os.path.join(dst_dir, "act_info.json")
    with open(path, "w") as f:
        json.dump(d, f)
    _CACHE["act_json"] = path
    os.environ["BASS_ACT_ROOT_JSON_PATH"] = path


def _fence(later_acts, earlier_act, reason):
    if not FENCES[0]:
        return
    for a in later_acts:
        add_dep_helper(a.ins, earlier_act.ins, sync=False, reason=reason)


def _build(stage=99):
    nc = bacc.Bacc("TRN2", target_bir_lowering=False, num_devices=N_CORES)

    xT = nc.dram_tensor("xT", [D, BS], F32, kind="ExternalInput")
    meansP = nc.dram_tensor("meansP", [K, D], F32, kind="ExternalInput")
    covsP = nc.dram_tensor("covsP", [K, D], F32, kind="ExternalInput")
    meansT = nc.dram_tensor("meansT", [D, K], F32, kind="ExternalInput")
    covsT = nc.dram_tensor("covsT", [D, K], F32, kind="ExternalInput")
    gamma128 = nc.dram_tensor("gamma128", [128, 1], F32, kind="ExternalInput")
    wmix = nc.dram_tensor("wmix", [1, KP], F32, kind="ExternalInput")
    W1T = nc.dram_tensor("W1T", [D, H0], F32R, kind="ExternalInput")
    W1sqT = nc.dram_tensor("W1sqT", [D, H0], BF16, kind="ExternalInput")
    W2T = nc.dram_tensor("W2T", [H0, H1], BF16, kind="ExternalInput")
    w3c = nc.dram_tensor("w3c", [128, H1C], F32R, kind="ExternalInput")
    b1c = nc.dram_tensor("b1c", [128, HC], F32, kind="ExternalInput")
    b2c = nc.dram_tensor("b2c", [128, H1C], F32, kind="ExternalInput")
    b3 = nc.dram_tensor("b3", [1, 1], F32, kind="ExternalInput")
    out = nc.dram_tensor("out", [1, BS], F32, kind="ExternalOutput")

    with tile.TileContext(nc) as tc:
        with (
            tc.tile_pool(name="wpool", bufs=1) as wpool,
            tc.tile_pool(name="data", bufs=1) as data,
            tc.tile_pool(name="small", bufs=2) as small,
            tc.tile_pool(name="stash", bufs=1) as stash,
            tc.tile_pool(name="transC", bufs=2) as trans,
            tc.tile_pool(name="psum", bufs=2, space="PSUM") as psum,
            tc.tile_pool(name="dram", bufs=2, space="DRAM") as dram,
        ):
            done = [False]

            def emit_dbg(pairs):
                dbg = small.tile([1, BS], F32, tag="outs", name="dbg")
                nc.vector.memset(dbg[:], 0.0)
                for off, ap in pairs:
                    nc.vector.tensor_copy(dbg[0:1, off:off + ap.shape[-1]], ap)
                nc.sync.dma_start(out[0:1, :], dbg[:])
                done[0] = True

            # ---------- tile allocs; x + small params DMA'd FIRST ----------
            w1t = [wpool.tile([128, H0], F32R, name=f"w1t{j}") for j in range(DC)]
            w1s = [wpool.tile([128, H0], BF16, name=f"w1s{j}") for j in range(DC)]
            w2t = [wpool.tile([128, H1], BF16, name=f"w2t{c}") for c in range(HC)]
            w3t = wpool.tile([128, H1C], F32R)
            b1t = wpool.tile([128, HC], F32)
            b2t = wpool.tile([128, H1C], F32)
            b3t = wpool.tile([1, 1], F32)
            meansPt = wpool.tile([K, D], F32)
            covsPt = wpool.tile([K, D], F32)
            mTt = wpool.tile([128, DC * K], F32)
            cTt = wpool.tile([128, DC * K], F32)
            g128 = wpool.tile([128, 1], F32)
            wmixt = wpool.tile([1, KP], F32)
            # critical-path inputs first: the q -> AllReduce chain needs only
            # xT + meansT/covsT/gamma/wmix; big weights overlap the collective
            for j in range(DC):
                nc.sync.dma_start(mTt[:, j * K:(j + 1) * K],
                                  meansT[j * 128:(j + 1) * 128, :])
                nc.sync.dma_start(cTt[:, j * K:(j + 1) * K],
                                  covsT[j * 128:(j + 1) * 128, :])
            nc.sync.dma_start(g128[:], gamma128[:])
            nc.sync.dma_start(wmixt[:], wmix[:])
            nc.sync.dma_start(meansPt[:], meansP[:])
            nc.sync.dma_start(covsPt[:], covsP[:])

            xm = [data.tile([128, BS], F32, name=f"xm{j}") for j in range(DC)]
            msk = [data.tile([128, BS], F32, name=f"msk{j}") for j in range(DC)]
            stats = data.tile([128, DC * 4], F32)
            mean1c = data.tile([128, DC], F32)
            covs1c = data.tile([128, DC], F32)
            init_acts = []

            # ---------- mask + per-column stats ----------
            stmp_cm = tc.tile_pool(name="statstmp", bufs=2)
            stmp = stmp_cm.__enter__()
            for j in range(DC):
                xc = stmp.tile([128, BS], F32, tag="xload", name=f"xc{j}")
                nc.sync.dma_start(xc[:], xT[j * 128:(j + 1) * 128, :])
                # msk holds the finite-mask first, inverted in place below
                a = nc.scalar.activation(msk[j][:], xc[:], AF.Is_finite,
                                         accum_out=stats[:, 4 * j:4 * j + 1])
                init_acts.append(a)
                nc.vector.memset(xm[j][:], 0.0)
                nc.vector.copy_predicated(
                    xm[j][:], msk[j][:].bitcast(mybir.dt.uint32), xc[:])
                nc.vector.tensor_reduce(stats[:, 4 * j + 1:4 * j + 2],
                                        xm[j][:], AX.X, ALU.add)
                # Square main output is scratch: dump it over xc
                a = nc.scalar.activation(xc[:], xm[j][:], AF.Square,
                                         accum_out=stats[:, 4 * j + 2:4 * j + 3])
                init_acts.append(a)
                nc.vector.tensor_scalar(msk[j][:], msk[j][:], -1.0, 1.0,
                                        ALU.mult, ALU.add)

            if stage <= 1:
                emit_dbg([(0, stats[0:1, 0:16])])

            # ---------- G matrices and partial q ----------
            if not done[0]:
                qps = psum.tile([1, KP], F32, tag="pre")
                us, Rs, Ls, msqs = [], [], [], []
                for j in range(DC):
                    cT_j = cTt[:, j * K:(j + 1) * K]
                    u = stmp.tile([128, K], F32, tag="gu", name=f"gu{j}",
                                  bufs=4)
                    nc.vector.tensor_scalar(u[:], cT_j, g128[:], None, ALU.add)
                    us.append(u)
                    R = stmp.tile([128, K], F32, tag="gr", name=f"gr{j}",
                                  bufs=4)
                    nc.vector.reciprocal(R[:], u[:])
                    Rs.append(R)
                    msq = stmp.tile([128, K], F32, tag="gm", name=f"gm{j}",
                                    bufs=4)
                    a = nc.scalar.activation(msq[:], mTt[:, j * K:(j + 1) * K],
                                             AF.Square)
                    init_acts.append(a)
                    msqs.append(msq)
                for j in range(DC):
                    L = stmp.tile([128, K], F32, tag="gl", name=f"gl{j}",
                                  bufs=4)
                    a = nc.scalar.activation(L[:], us[j][:], AF.Ln)
                    init_acts.append(a)
                    Ls.append(L)
                n_mm = 0
                for j in range(DC):
                    mT_j = mTt[:, j * K:(j + 1) * K]
                    G0 = stmp.tile([128, K], F32, tag="g0", name=f"g0{j}")
                    nc.vector.tensor_tensor(G0[:], msqs[j][:], Rs[j][:],
                                            ALU.mult)
                    nc.vector.tensor_tensor(G0[:], G0[:], Ls[j][:], ALU.add)
                    nc.vector.tensor_scalar(G0[:], G0[:], LOG_2PI, None,
                                            ALU.add)
                    G1 = stmp.tile([128, K], F32, tag="g1", name=f"g1{j}")
                    nc.vector.scalar_tensor_tensor(G1[:], mT_j, -2.0, Rs[j][:],
                                                   ALU.mult, ALU.mult)
                    for col, G in ((0, G0), (1, G1), (2, Rs[j])):
                        nc.tensor.matmul(qps[0:1, 0:K],
                                         stats[:, 4 * j + col:4 * j + col + 1],
                                         G[:], start=(n_mm == 0),
                                         stop=(n_mm == 11))
                        n_mm += 1
                qsb = small.tile([1, KP], F32)
                nc.vector.memset(qsb[:], 0.0)
                nc.vector.tensor_copy(qsb[0:1, 0:K], qps[0:1, 0:K])
                if stage <= 2:
                    emit_dbg([(0, qsb[0:1, :])])

            # ---------- AllReduce partial q ----------
            if not done[0]:
                cin = dram.tile([1, KP], F32)
                cout = dram.tile([1, KP], F32, addr_space="Shared")
                nc.gpsimd.dma_start(cin[:], qsb[:])
                nc.gpsimd.collective_compute(
                    "AllReduce", ALU.add,
                    ins=[cin[:].opt()], outs=[cout[:].opt()],
                    replica_groups=[list(range(N_CORES))],
                )
                qg = small.tile([1, KP], F32)
                nc.sync.dma_start(qg[:], cout[:])

                # big weights: issued here so they stream during the AllReduce
                for j in range(DC):
                    nc.sync.dma_start(w1t[j][:], W1T[j * 128:(j + 1) * 128, :])
                    nc.sync.dma_start(w1s[j][:], W1sqT[j * 128:(j + 1) * 128, :])
                for c in range(HC):
                    nc.sync.dma_start(w2t[c][:], W2T[c * 128:(c + 1) * 128, :])
                nc.sync.dma_start(w3t[:], w3c[:])
                nc.sync.dma_start(b1t[:], b1c[:])
                nc.sync.dma_start(b2t[:], b2c[:])
                nc.sync.dma_start(b3t[:], b3[:])
                if stage <= 3:
                    emit_dbg([(0, qg[0:1, :])])

            # ---------- log_p, log_q, r, selection ----------
            if not done[0]:
                mx = small.tile([1, 1], F32)
                nc.vector.tensor_reduce(mx[:], wmixt[:], AX.X, ALU.max)
                nmx = small.tile([1, 1], F32)
                nc.vector.tensor_scalar(nmx[:], mx[:], -1.0, None, ALU.mult)
                ew = small.tile([1, KP], F32)
                a = nc.scalar.activation(ew[:], wmixt[:], AF.Exp, bias=nmx[:])
                init_acts.append(a)
                sw = small.tile([1, 1], F32)
                nc.vector.tensor_reduce(sw[:], ew[:], AX.X, ALU.add)
                lsw = small.tile([1, 1], F32)
                a = nc.scalar.activation(lsw[:], sw[:], AF.Ln)
                init_acts.append(a)
                nlsw = small.tile([1, 1], F32)
                nc.vector.tensor_scalar(nlsw[:], lsw[:], -1.0, None, ALU.mult)
                logp = small.tile([1, KP], F32)
                nc.vector.tensor_scalar(logp[:], wmixt[:], nmx[:], nlsw[:],
                                        ALU.add, ALU.add)
                # LOCAL log_q: selection runs speculatively on this core's
                # shard-local q. The AllReduce proceeds concurrently and the
                # output is multiplied by 1/agree at the end (inf if the
                # global argmax/softmax ever disagrees with the local one).
                lq = small.tile([1, KP], F32)
                nc.vector.scalar_tensor_tensor(lq[:], qsb[:], -0.5, logp[:],
                                               ALU.mult, ALU.add)
                mx2 = small.tile([1, 1], F32)
                nc.vector.tensor_reduce(mx2[:], lq[:], AX.X, ALU.max)
                nmx2 = small.tile([1, 1], F32)
                nc.vector.tensor_scalar(nmx2[:], mx2[:], -1.0, None, ALU.mult)
                er = small.tile([1, KP], F32)
                a = nc.scalar.activation(er[:], lq[:], AF.Exp, bias=nmx2[:])
                init_acts.append(a)
                sre = small.tile([1, 1], F32)
                nc.vector.tensor_reduce(sre[:], er[:], AX.X, ALU.add)
                isr = small.tile([1, 1], F32)
                nc.vector.reciprocal(isr[:], sre[:])
                rvec = small.tile([1, KP], F32)
                nc.vector.tensor_scalar(rvec[:], er[:], isr[:], None, ALU.mult)
                r1 = small.tile([1, KP], F32)
                nc.vector.tensor_scalar(r1[:], lq[:], mx2[:], None,
                                        ALU.is_equal)
                rv1 = small.tile([1, 1], F32)
                nc.vector.tensor_reduce(rv1[:], rvec[:], AX.X, ALU.max)
                lr = small.tile([1, 1], F32)
                a = nc.scalar.activation(lr[:], rv1[:], AF.Ln)
                init_acts.append(a)
                lcr = small.tile([1, 1], F32)
                nc.vector.tensor_scalar(lcr[:], lr[:],
                                        float(np.log(INV_SQRT_2PI)), None,
                                        ALU.add)
                lr128 = small.tile([128, 1], F32, bufs=1)
                lcr128 = small.tile([128, 1], F32, bufs=1)
                if PBCAST[0]:
                    nc.gpsimd.partition_broadcast(lr128[:], lr[:], 128)
                    nc.gpsimd.partition_broadcast(lcr128[:], lcr[:], 128)
                else:
                    nc.gpsimd.dma_start(
                        lr128[:], lr[0:1, 0:1].broadcast_to([128, 1]))
                    nc.gpsimd.dma_start(
                        lcr128[:], lcr[0:1, 0:1].broadcast_to([128, 1]))

                ones11 = small.tile([1, 1], F32)
                nc.vector.memset(ones11[:], 1.0)
                r1Tp = psum.tile([K, 1], F32, tag="pre")
                nc.tensor.matmul(r1Tp[:], r1[0:1, 0:K], ones11[:],
                                 start=True, stop=True)
                r1Ts = small.tile([K, 1], F32)
                nc.vector.tensor_copy(r1Ts[:], r1Tp[:])
                for c in range(DC):
                    mp = psum.tile([128, 1], F32, tag="pre", name=f"mp{c}")
                    nc.tensor.matmul(mp[:], meansPt[:, c * 128:(c + 1) * 128],
                                     r1Ts[:], start=True, stop=True)
                    nc.vector.tensor_copy(mean1c[:, c:c + 1], mp[:])
                    cp = psum.tile([128, 1], F32, tag="sig", name=f"cp{c}")
                    nc.tensor.matmul(cp[:], covsPt[:, c * 128:(c + 1) * 128],
                                     r1Ts[:], start=True, stop=True)
                    nc.vector.tensor_copy(covs1c[:, c:c + 1], cp[:])
                # global agreement check: floored to the end of the
                # scheduler clock so the AR-dependent ops sit at the TAIL of
                # every engine stream (the AR itself runs concurrently with
                # the main loop).
                with tc.tile_wait_until(50):
                    lqg = small.tile([1, KP], F32)
                    nc.vector.scalar_tensor_tensor(lqg[:], qg[:], -0.5,
                                                   logp[:], ALU.mult, ALU.add)
                    mxg = small.tile([1, 1], F32)
                    nc.vector.tensor_reduce(mxg[:], lqg[:], AX.X, ALU.max)
                    nmxg = small.tile([1, 1], F32)
                    nc.vector.tensor_scalar(nmxg[:], mxg[:], -1.0, None,
                                            ALU.mult)
                    r1g = small.tile([1, KP], F32)
                    nc.vector.tensor_scalar(r1g[:], lqg[:], mxg[:], None,
                                            ALU.is_equal)
                    agv = small.tile([1, KP], F32)
                    nc.vector.tensor_tensor(agv[:], r1g[:], r1[:], ALU.mult)
                    agr = small.tile([1, 1], F32)
                    nc.vector.tensor_reduce(agr[:], agv[:], AX.X, ALU.add)
                    nc.vector.tensor_scalar(agr[:], agr[:], 1.0, None, ALU.min)
                    eg = small.tile([1, KP], F32)
                    nc.scalar.activation(eg[:], lqg[:], AF.Exp, bias=nmxg[:])
                    sg = small.tile([1, 1], F32)
                    nc.vector.tensor_reduce(sg[:], eg[:], AX.X, ALU.add)
                    rvg = small.tile([1, 1], F32)
                    nc.vector.reciprocal(rvg[:], sg[:])
                    dv = small.tile([1, 1], F32)
                    nc.vector.tensor_tensor(dv[:], rv1[:], rvg[:],
                                            ALU.subtract)
                    nc.vector.tensor_tensor(dv[:], dv[:], dv[:], ALU.mult)
                    ok2 = small.tile([1, 1], F32)
                    nc.vector.tensor_scalar(ok2[:], dv[:], 1.0e-6, None,
                                            ALU.is_lt)
                    flag = small.tile([1, 1], F32)
                    nc.vector.tensor_tensor(flag[:], agr[:], ok2[:], ALU.mult)
                    finv = small.tile([1, 1], F32, bufs=1)
                    nc.vector.reciprocal(finv[:], flag[:])
                if stage <= 4:
                    emit_dbg([(0, rvec[0:1, :]), (KP, rv1[0:1, :]),
                              (32, mean1c[0:1, :]), (40, covs1c[0:1, :]),
                              (48, flag[0:1, :])])

            stmp_cm.__exit__(None, None, None)

            # ---------- main loop ----------
            # ACT-stream ordering: Ln and Exp land in different walrus table
            # sets, so every Ln<->Exp boundary costs a ~1.3us ACT_TABLE_LOAD.
            # Sigma is staged PSUM->SBUF by the DVE so all 8 Ln's of a batch
            # tile run as one block; scheduler-clock floors (tile_wait_until)
            # are applied ONLY to ACT instructions, ordering the ACT stream
            # as [Ln x8][Exp x24 + universal][Gelu x8] per tile while leaving
            # PE/DVE/DMA free to overlap.
            if not done[0]:
                sigp_cm = tc.tile_pool(name="sigpool", bufs=1)
                sigpool = sigp_cm.__enter__()
                hmix_t, outs_t = {}, {}

                def stage_a(t):
                    bsl = slice(t * BT, (t + 1) * BT)
                    fb = 10.0 * (t + 1)
                    dm = [stash.tile([128, BT], F32R, tag=f"dm{j}",
                                     name=f"dm{t}_{j}") for j in range(DC)]
                    mc = [stash.tile([128, BT], BF16, tag=f"mc{j}",
                                     name=f"mc{t}_{j}") for j in range(DC)]
                    for j in range(DC):
                        nc.vector.scalar_tensor_tensor(
                            dm[j][:], msk[j][:, bsl], mean1c[:, j:j + 1],
                            xm[j][:, bsl], ALU.mult, ALU.add)
                        nc.vector.tensor_scalar(
                            mc[j][:], msk[j][:, bsl], covs1c[:, j:j + 1],
                            None, ALU.mult)

                    sigS = []
                    for hc in range(HC):
                        hsl = slice(hc * 128, (hc + 1) * 128)
                        sig = psum.tile([128, BT], F32, tag="sig",
                                        name=f"sig{t}_{hc}")
                        for j in range(DC):
                            nc.tensor.matmul(sig[:], w1s[j][:, hsl],
                                             mc[j][:], start=(j == 0),
                                             stop=(j == DC - 1))
                        ss = sigpool.tile([128, BT], F32, tag=f"ss{hc}",
                                          name=f"ss{t}_{hc}", bufs=2)
                        nc.vector.tensor_copy(ss[:], sig[:])
                        sigS.append(ss)
                    wst = []
                    for hc in range(HC):
                        hsl = slice(hc * 128, (hc + 1) * 128)
                        pre = psum.tile([128, BT], F32, tag="pre",
                                        name=f"pre{t}_{hc}")
                        for j in range(DC):
                            nc.tensor.matmul(pre[:], w1t[j][:, hsl],
                                             dm[j][:], start=(j == 0),
                                             stop=(j == DC - 1))
                        # wt_ holds mu = relu(pre + b1) for now; becomes
                        # w = mu/sqrt(sig) in place after rs is ready
                        wt_ = stash.tile([128, BT], F32, tag=f"wt{hc}",
                                         name=f"wt{t}_{hc}")
                        nc.vector.tensor_scalar(wt_[:], pre[:],
                                                b1t[:, hc:hc + 1], 0.0,
                                                ALU.add, ALU.max)
                        wst.append(wt_)

                    # Ln block (one table load); in place over staged sigma
                    with tc.tile_wait_until(fb):
                        for hc in range(HC):
                            nc.scalar.activation(sigS[hc][:], sigS[hc][:],
                                                 AF.Ln)

                    # Exp block (one table load); Square is set-universal
                    srst, hmix = [], []
                    for hc in range(HC):
                        tl = sigS[hc]
                        with tc.tile_wait_until(fb + 1):
                            rs = trans.tile([128, BT], F32, tag="w2",
                                            name=f"rs{t}_{hc}")
                            nc.scalar.activation(rs[:], tl[:], AF.Exp,
                                                 scale=-0.5)
                            srw = stash.tile([128, BT], F32, tag=f"srw{hc}",
                                             name=f"srw{t}_{hc}", bufs=2)
                            nc.scalar.activation(srw[:], tl[:], AF.Exp,
                                                 scale=0.5, bias=lr128[:])
                        nc.vector.tensor_tensor(wst[hc][:], wst[hc][:],
                                                rs[:], ALU.mult)
                        w2 = trans.tile([128, BT], F32, tag="w2",
                                        name=f"w2{t}_{hc}")
                        with tc.tile_wait_until(fb + 1):
                            nc.scalar.activation(w2[:], wst[hc][:], AF.Square)
                        # z = t - w^2, in place over w2
                        nc.vector.tensor_tensor(w2[:], tl[:], w2[:],
                                                ALU.subtract)
                        ce = trans.tile([128, BT], F32, tag="ce",
                                        name=f"ce{t}_{hc}")
                        with tc.tile_wait_until(fb + 1):
                            nc.scalar.activation(ce[:], w2[:], AF.Exp,
                                                 scale=0.5, bias=lcr128[:])
                        srst.append(srw)
                        hmix.append(ce)

                    # Gelu block (one table load); in place over w.
                    # hm = ce + srw*gelu(w) written once as bf16 (single
                    # rounding) for the bf16 layer-2 matmul.
                    hms = []
                    for hc in range(HC):
                        with tc.tile_wait_until(fb + 2):
                            nc.scalar.activation(wst[hc][:], wst[hc][:],
                                                 AF.Gelu)
                        nc.vector.tensor_tensor(wst[hc][:], srst[hc][:],
                                                wst[hc][:], ALU.mult)
                        hm = stash.tile([128, BT], BF16, tag=f"hm{hc}",
                                        name=f"hm{t}_{hc}", bufs=2)
                        nc.vector.tensor_tensor(hm[:], hmix[hc][:],
                                                wst[hc][:], ALU.add)
                        hms.append(hm)
                    hmix_t[t] = hms

                def stage_l(t):
                    bsl = slice(t * BT, (t + 1) * BT)
                    fb = 10.0 * (t + 1)
                    hmix = hmix_t[t]
                    h2s = []
                    for half in range(2):
                        h2ps = [psum.tile([128, BT], F32, tag="h2p", bufs=4,
                                          name=f"h2p{t}_{half}_{i}")
                                for i in range(4)]
                        for c0 in range(HC):
                            for i in range(4):
                                c1 = half * 4 + i
                                nc.tensor.matmul(
                                    h2ps[i][:],
                                    w2t[c0][:, c1 * 128:(c1 + 1) * 128],
                                    hmix[c0][:], start=(c0 == 0),
                                    stop=(c0 == HC - 1))
                        for i in range(4):
                            c1 = half * 4 + i
                            h2 = trans.tile([128, BT], F32R, tag="h2",
                                            name=f"h2{t}_{c1}")
                            with tc.tile_wait_until(fb + 2):
                                nc.scalar.activation(h2[:], h2ps[i][:],
                                                     AF.Prelu,
                                                     bias=b2t[:, c1:c1 + 1],
                                                     alpha=0.2)
                            h2s.append(h2)
                    outp = psum.tile([1, BT], F32, tag="sig",
                                     name=f"outp{t}")
                    for c1 in range(H1C):
                        nc.tensor.matmul(outp[:], w3t[:, c1:c1 + 1],
                                         h2s[c1][:], start=(c1 == 0),
                                         stop=(c1 == H1C - 1))
                    with tc.tile_wait_until(51):
                        outs = small.tile([1, BT], F32, tag="outs",
                                          name=f"outs{t}")
                        nc.vector.tensor_scalar(outs[:], outp[:], b3t[:],
                                                finv[:], ALU.add, ALU.mult)
                        nc.sync.dma_start(out[0:1, bsl], outs[:])

                # A(0) A(1) L(0) L(1): the t=1 layer-1 matmuls keep the PE
                # busy during t=0's gelu phase, and L2(0) fills t=1's.
                stage_a(0)
                stage_a(1)
                stage_l(0)
                stage_l(1)
                sigp_cm.__exit__(None, None, None)

    nc.compile()
    return nc


def _prep_inputs(x, means, covs, weights_mix, gamma, W1, b1, W2, b2, W3, b3):
    x = np.asarray(x, np.float32)
    means = np.ascontiguousarray(np.asarray(means, np.float32))
    covs = np.ascontiguousarray(np.asarray(covs, np.float32))
    W1 = np.asarray(W1, np.float32)
    W2 = np.asarray(W2, np.float32)
    W3 = np.asarray(W3, np.float32)
    wmixv = np.full((1, KP), NEG_BIG, np.float32)
    wmixv[0, :K] = np.asarray(weights_mix, np.float32)
    common = {
        "meansP": means,
        "covsP": covs,
        "meansT": np.ascontiguousarray(means.T),
        "covsT": np.ascontiguousarray(covs.T),
        "gamma128": np.full((128, 1), np.float32(gamma), np.float32),
        "wmix": wmixv,
        "W1T": np.ascontiguousarray(W1.T),
        "W1sqT": np.ascontiguousarray((W1 * W1).T).astype(ml_dtypes.bfloat16),
        "W2T": np.ascontiguousarray(W2.T).astype(ml_dtypes.bfloat16),
        "w3c": np.ascontiguousarray(W3.reshape(H1C, 128).T),
        "b1c": np.ascontiguousarray(np.asarray(b1, np.float32).reshape(HC, 128).T),
        "b2c": np.ascontiguousarray(np.asarray(b2, np.float32).reshape(H1C, 128).T),
        "b3": np.asarray(b3, np.float32).reshape(1, 1),
    }
    in_maps = []
    for i in range(N_CORES):
        shard = x[i * BS:(i + 1) * BS]
        m = dict(common)
        m["xT"] = np.ascontiguousarray(shard.T)
        in_maps.append(m)
    return in_maps


def kernel(**inputs):
    if "nc" not in _CACHE:
        _CACHE["nc"] = _build()
    nc = _CACHE["nc"]
    in_maps = _prep_inputs(**inputs)
    res = run_bass_kernel_spmd(nc, in_maps, list(range(N_CORES)))
    parts = [res.results[i]["out"][0] for i in range(N_CORES)]
    return np.concatenate(parts).reshape(B, 1).astype(np.float32)


# revision 23
# speedup vs baseline: 1.0444x; 1.0444x over previous
"""Trainium2 Bass kernel for nn_DiscriminatorForMissing (NaN branch).

Data-parallel over batch: each of 8 cores gets B/8 = 1024 rows, with the
per-distribution log-likelihoods q[k] AllReduced across cores before the
softmax mixing.

Math notes (m = isnan(x), xm = where(m, 0, x)):
  q[k] reduces exactly to masked per-column batch stats (cnt, Sx, Sxx):
    q[k] = sum_d cnt[d]*(ln(g+c_kd) + means_kd^2/(g+c_kd) + LOG_2PI)
         + sum_d Sxx[d]/(g+c_kd) - 2*sum_d Sx[d]*means_kd/(g+c_kd)
  r = softmax(-q/2 + log_p) is numerically one-hot for any realistic
  input (|q| gaps ~1e5 >> fp32 softmax range), so the K-mixture collapses
  to the argmax distribution: we select means[k*]/covs[k*] with a matmul
  against the one-hot mask (r == max(r)) and scale by the true r_k*.

  relu-moment: nr(mu, sig) = s*phi(w) + mu*Phi(w), w = mu/s, s = sqrt(sig)
    = exp(0.5*(ln sig - w^2) + ln(c1*r)) + exp(0.5*ln sig + ln r)*Gelu(w)
  -> only {Ln, Exp} + {Gelu} ACT table sets; phases fenced to avoid
  table-set thrashing.

All heavy matmuls run as float32r (1 cyc/row, ~13-bit mantissa).
"""

import os
import sys
import tempfile

import numpy as np
import ml_dtypes

if "/opt/trn_rl_repo" not in sys.path:
    sys.path.insert(0, "/opt/trn_rl_repo")

import concourse.bass as bass  # noqa: E402,F401
import concourse.tile as tile  # noqa: E402
from concourse import mybir, bacc  # noqa: E402
from concourse.bass_utils import run_bass_kernel_spmd  # noqa: E402
from bass_rust import add_dep_helper  # noqa: E402

B, D, H0, H1, K = 8192, 512, 1024, 1024, 10
N_CORES = 8
BS = B // N_CORES          # rows per core
BT = 512                   # batch tile (free dim)
NBT = BS // BT             # 2 batch tiles
DC = D // 128              # 4 contraction chunks
HC = H0 // 128             # 8 hidden chunks
H1C = H1 // 128            # 8 layer-2 chunks
KP = 16                    # padded K for [1,16] vectors

LOG_2PI = float(np.log(2.0 * np.pi))
INV_SQRT_2PI = 0.3989422804014327
NEG_BIG = -1.0e30

F32 = mybir.dt.float32
F32R = mybir.dt.float32r
BF16 = mybir.dt.bfloat16
AF = mybir.ActivationFunctionType
ALU = mybir.AluOpType
AX = mybir.AxisListType

_CACHE = {}
FENCES = [True]
PBCAST = [True]


def _setup_act_tables():
    """Point walrus at a reordered act_info.json so Ln and Exp both resolve
    to natural_log_exp_and_others (one ACT table set instead of two, which
    otherwise costs a ~1.3us ACT_TABLE_LOAD at every Ln<->Exp boundary)."""
    if "act_json" in _CACHE:
        os.environ["BASS_ACT_ROOT_JSON_PATH"] = _CACHE["act_json"]
        return
    import json
    import neuronxcc
    src_dir = os.path.join(os.path.dirname(neuronxcc.__file__),
                           "pwp", "pwp_bin_trainium")
    dst_dir = os.path.join(tempfile.mkdtemp(prefix="act_pwp_"), "pwp")
    os.makedirs(dst_dir, exist_ok=True)
    for f in os.listdir(src_dir):
        if f != "act_info.json":
            os.symlink(os.path.join(src_dir, f), os.path.join(dst_dir, f))
    d = json.load(open(os.path.join(src_dir, "act_info.json")))
    sets = d["act_func_sets"]
    sets.sort(key=lambda s: 0 if s["name"] == "natural_log_exp_and_others" else 1)
    path = os.path.join(dst_dir, "act_info.json")
    with open(path, "w") as f:
        json.dump(d, f)
    _CACHE["act_json"] = path
    os.environ["BASS_ACT_ROOT_JSON_PATH"] = path


def _fence(later_acts, earlier_act, reason):
    if not FENCES[0]:
        return
    for a in later_acts:
        add_dep_helper(a.ins, earlier_act.ins, sync=False, reason=reason)


def _build(stage=99):
    nc = bacc.Bacc("TRN2", target_bir_lowering=False, num_devices=N_CORES)

    xT = nc.dram_tensor("xT", [D, BS], F32, kind="ExternalInput")
    meansP = nc.dram_tensor("meansP", [K, D], F32, kind="ExternalInput")
    covsP = nc.dram_tensor("covsP", [K, D], F32, kind="ExternalInput")
    meansT = nc.dram_tensor("meansT", [D, K], F32, kind="ExternalInput")
    covsT = nc.dram_tensor("covsT", [D, K], F32, kind="ExternalInput")
    gamma128 = nc.dram_tensor("gamma128", [128, 1], F32, kind="ExternalInput")
    wmix = nc.dram_tensor("wmix", [1, KP], F32, kind="ExternalInput")
    W1T = nc.dram_tensor("W1T", [D, H0], F32R, kind="ExternalInput")
    W1sqT = nc.dram_tensor("W1sqT", [D, H0], BF16, kind="ExternalInput")
    W2T = nc.dram_tensor("W2T", [H0, H1], F32R, kind="ExternalInput")
    w3c = nc.dram_tensor("w3c", [128, H1C], F32R, kind="ExternalInput")
    b1c = nc.dram_tensor("b1c", [128, HC], F32, kind="ExternalInput")
    b2c = nc.dram_tensor("b2c", [128, H1C], F32, kind="ExternalInput")
    b3 = nc.dram_tensor("b3", [1, 1], F32, kind="ExternalInput")
    out = nc.dram_tensor("out", [1, BS], F32, kind="ExternalOutput")

    with tile.TileContext(nc) as tc:
        with (
            tc.tile_pool(name="wpool", bufs=1) as wpool,
            tc.tile_pool(name="data", bufs=1) as data,
            tc.tile_pool(name="small", bufs=2) as small,
            tc.tile_pool(name="stash", bufs=1) as stash,
            tc.tile_pool(name="transC", bufs=2) as trans,
            tc.tile_pool(name="psum", bufs=2, space="PSUM") as psum,
            tc.tile_pool(name="dram", bufs=2, space="DRAM") as dram,
        ):
            done = [False]

            def emit_dbg(pairs):
                dbg = small.tile([1, BS], F32, tag="outs", name="dbg")
                nc.vector.memset(dbg[:], 0.0)
                for off, ap in pairs:
                    nc.vector.tensor_copy(dbg[0:1, off:off + ap.shape[-1]], ap)
                nc.sync.dma_start(out[0:1, :], dbg[:])
                done[0] = True

            # ---------- tile allocs; x + small params DMA'd FIRST ----------
            w1t = [wpool.tile([128, H0], F32R, name=f"w1t{j}") for j in range(DC)]
            w1s = [wpool.tile([128, H0], BF16, name=f"w1s{j}") for j in range(DC)]
            w2t = [wpool.tile([128, H1], F32R, name=f"w2t{c}") for c in range(HC)]
            w3t = wpool.tile([128, H1C], F32R)
            b1t = wpool.tile([128, HC], F32)
            b2t = wpool.tile([128, H1C], F32)
            b3t = wpool.tile([1, 1], F32)
            meansPt = wpool.tile([K, D], F32)
            covsPt = wpool.tile([K, D], F32)
            mTt = wpool.tile([128, DC * K], F32)
            cTt = wpool.tile([128, DC * K], F32)
            g128 = wpool.tile([128, 1], F32)
            wmixt = wpool.tile([1, KP], F32)
            # critical-path inputs first: the q -> AllReduce chain needs only
            # xT + meansT/covsT/gamma/wmix; big weights overlap the collective
            for j in range(DC):
                nc.sync.dma_start(mTt[:, j * K:(j + 1) * K],
                                  meansT[j * 128:(j + 1) * 128, :])
                nc.sync.dma_start(cTt[:, j * K:(j + 1) * K],
                                  covsT[j * 128:(j + 1) * 128, :])
            nc.sync.dma_start(g128[:], gamma128[:])
            nc.sync.dma_start(wmixt[:], wmix[:])
            nc.sync.dma_start(meansPt[:], meansP[:])
            nc.sync.dma_start(covsPt[:], covsP[:])

            xm = [data.tile([128, BS], F32, name=f"xm{j}") for j in range(DC)]
            msk = [data.tile([128, BS], F32, name=f"msk{j}") for j in range(DC)]
            stats = data.tile([128, DC * 4], F32)
            mean1c = data.tile([128, DC], F32)
            covs1c = data.tile([128, DC], F32)
            init_acts = []

            # ---------- mask + per-column stats ----------
            stmp_cm = tc.tile_pool(name="statstmp", bufs=2)
            stmp = stmp_cm.__enter__()
            for j in range(DC):
                xc = stmp.tile([128, BS], F32, tag="xload", name=f"xc{j}")
                nc.sync.dma_start(xc[:], xT[j * 128:(j + 1) * 128, :])
                # msk holds the finite-mask first, inverted in place below
                a = nc.scalar.activation(msk[j][:], xc[:], AF.Is_finite,
                                         accum_out=stats[:, 4 * j:4 * j + 1])
                init_acts.append(a)
                nc.vector.memset(xm[j][:], 0.0)
                nc.vector.copy_predicated(
                    xm[j][:], msk[j][:].bitcast(mybir.dt.uint32), xc[:])
                nc.vector.tensor_reduce(stats[:, 4 * j + 1:4 * j + 2],
                                        xm[j][:], AX.X, ALU.add)
                # Square main output is scratch: dump it over xc
                a = nc.scalar.activation(xc[:], xm[j][:], AF.Square,
                                         accum_out=stats[:, 4 * j + 2:4 * j + 3])
                init_acts.append(a)
                nc.vector.tensor_scalar(msk[j][:], msk[j][:], -1.0, 1.0,
                                        ALU.mult, ALU.add)

            if stage <= 1:
                emit_dbg([(0, stats[0:1, 0:16])])

            # ---------- G matrices and partial q ----------
            if not done[0]:
                qps = psum.tile([1, KP], F32, tag="pre")
                us, Rs, Ls, msqs = [], [], [], []
                for j in range(DC):
                    cT_j = cTt[:, j * K:(j + 1) * K]
                    u = stmp.tile([128, K], F32, tag="gu", name=f"gu{j}",
                                  bufs=4)
                    nc.vector.tensor_scalar(u[:], cT_j, g128[:], None, ALU.add)
                    us.append(u)
                    R = stmp.tile([128, K], F32, tag="gr", name=f"gr{j}",
                                  bufs=4)
                    nc.vector.reciprocal(R[:], u[:])
                    Rs.append(R)
                    msq = stmp.tile([128, K], F32, tag="gm", name=f"gm{j}",
                                    bufs=4)
                    a = nc.scalar.activation(msq[:], mTt[:, j * K:(j + 1) * K],
                                             AF.Square)
                    init_acts.append(a)
                    msqs.append(msq)
                for j in range(DC):
                    L = stmp.tile([128, K], F32, tag="gl", name=f"gl{j}",
                                  bufs=4)
                    a = nc.scalar.activation(L[:], us[j][:], AF.Ln)
                    init_acts.append(a)
                    Ls.append(L)
                n_mm = 0
                for j in range(DC):
                    mT_j = mTt[:, j * K:(j + 1) * K]
                    G0 = stmp.tile([128, K], F32, tag="g0", name=f"g0{j}")
                    nc.vector.tensor_tensor(G0[:], msqs[j][:], Rs[j][:],
                                            ALU.mult)
                    nc.vector.tensor_tensor(G0[:], G0[:], Ls[j][:], ALU.add)
                    nc.vector.tensor_scalar(G0[:], G0[:], LOG_2PI, None,
                                            ALU.add)
                    G1 = stmp.tile([128, K], F32, tag="g1", name=f"g1{j}")
                    nc.vector.scalar_tensor_tensor(G1[:], mT_j, -2.0, Rs[j][:],
                                                   ALU.mult, ALU.mult)
                    for col, G in ((0, G0), (1, G1), (2, Rs[j])):
                        nc.tensor.matmul(qps[0:1, 0:K],
                                         stats[:, 4 * j + col:4 * j + col + 1],
                                         G[:], start=(n_mm == 0),
                                         stop=(n_mm == 11))
                        n_mm += 1
                qsb = small.tile([1, KP], F32)
                nc.vector.memset(qsb[:], 0.0)
                nc.vector.tensor_copy(qsb[0:1, 0:K], qps[0:1, 0:K])
                if stage <= 2:
                    emit_dbg([(0, qsb[0:1, :])])

            # ---------- AllReduce partial q ----------
            if not done[0]:
                cin = dram.tile([1, KP], F32)
                cout = dram.tile([1, KP], F32, addr_space="Shared")
                nc.gpsimd.dma_start(cin[:], qsb[:])
                nc.gpsimd.collective_compute(
                    "AllReduce", ALU.add,
                    ins=[cin[:].opt()], outs=[cout[:].opt()],
                    replica_groups=[list(range(N_CORES))],
                )
                qg = small.tile([1, KP], F32)
                nc.sync.dma_start(qg[:], cout[:])

                # big weights: issued here so they stream during the AllReduce
                for j in range(DC):
                    nc.sync.dma_start(w1t[j][:], W1T[j * 128:(j + 1) * 128, :])
                    nc.sync.dma_start(w1s[j][:], W1sqT[j * 128:(j + 1) * 128, :])
                for c in range(HC):
                    nc.sync.dma_start(w2t[c][:], W2T[c * 128:(c + 1) * 128, :])
                nc.sync.dma_start(w3t[:], w3c[:])
                nc.sync.dma_start(b1t[:], b1c[:])
                nc.sync.dma_start(b2t[:], b2c[:])
                nc.sync.dma_start(b3t[:], b3[:])
                if stage <= 3:
                    emit_dbg([(0, qg[0:1, :])])

            # ---------- log_p, log_q, r, selection ----------
            if not done[0]:
                mx = small.tile([1, 1], F32)
                nc.vector.tensor_reduce(mx[:], wmixt[:], AX.X, ALU.max)
                nmx = small.tile([1, 1], F32)
                nc.vector.tensor_scalar(nmx[:], mx[:], -1.0, None, ALU.mult)
                ew = small.tile([1, KP], F32)
                a = nc.scalar.activation(ew[:], wmixt[:], AF.Exp, bias=nmx[:])
                init_acts.append(a)
                sw = small.tile([1, 1], F32)
                nc.vector.tensor_reduce(sw[:], ew[:], AX.X, ALU.add)
                lsw = small.tile([1, 1], F32)
                a = nc.scalar.activation(lsw[:], sw[:], AF.Ln)
                init_acts.append(a)
                nlsw = small.tile([1, 1], F32)
                nc.vector.tensor_scalar(nlsw[:], lsw[:], -1.0, None, ALU.mult)
                logp = small.tile([1, KP], F32)
                nc.vector.tensor_scalar(logp[:], wmixt[:], nmx[:], nlsw[:],
                                        ALU.add, ALU.add)
                # LOCAL log_q: selection runs speculatively on this core's
                # shard-local q. The AllReduce proceeds concurrently and the
                # output is multiplied by 1/agree at the end (inf if the
                # global argmax/softmax ever disagrees with the local one).
                lq = small.tile([1, KP], F32)
                nc.vector.scalar_tensor_tensor(lq[:], qsb[:], -0.5, logp[:],
                                               ALU.mult, ALU.add)
                mx2 = small.tile([1, 1], F32)
                nc.vector.tensor_reduce(mx2[:], lq[:], AX.X, ALU.max)
                nmx2 = small.tile([1, 1], F32)
                nc.vector.tensor_scalar(nmx2[:], mx2[:], -1.0, None, ALU.mult)
                er = small.tile([1, KP], F32)
                a = nc.scalar.activation(er[:], lq[:], AF.Exp, bias=nmx2[:])
                init_acts.append(a)
                sre = small.tile([1, 1], F32)
                nc.vector.tensor_reduce(sre[:], er[:], AX.X, ALU.add)
                isr = small.tile([1, 1], F32)
                nc.vector.reciprocal(isr[:], sre[:])
                rvec = small.tile([1, KP], F32)
                nc.vector.tensor_scalar(rvec[:], er[:], isr[:], None, ALU.mult)
                r1 = small.tile([1, KP], F32)
                nc.vector.tensor_scalar(r1[:], lq[:], mx2[:], None,
                                        ALU.is_equal)
                rv1 = small.tile([1, 1], F32)
                nc.vector.tensor_reduce(rv1[:], rvec[:], AX.X, ALU.max)
                lr = small.tile([1, 1], F32)
                a = nc.scalar.activation(lr[:], rv1[:], AF.Ln)
                init_acts.append(a)
                lcr = small.tile([1, 1], F32)
                nc.vector.tensor_scalar(lcr[:], lr[:],
                                        float(np.log(INV_SQRT_2PI)), None,
                                        ALU.add)
                lr128 = small.tile([128, 1], F32, bufs=1)
                lcr128 = small.tile([128, 1], F32, bufs=1)
                if PBCAST[0]:
                    nc.gpsimd.partition_broadcast(lr128[:], lr[:], 128)
                    nc.gpsimd.partition_broadcast(lcr128[:], lcr[:], 128)
                else:
                    nc.gpsimd.dma_start(
                        lr128[:], lr[0:1, 0:1].broadcast_to([128, 1]))
                    nc.gpsimd.dma_start(
                        lcr128[:], lcr[0:1, 0:1].broadcast_to([128, 1]))

                ones11 = small.tile([1, 1], F32)
                nc.vector.memset(ones11[:], 1.0)
                r1Tp = psum.tile([K, 1], F32, tag="pre")
                nc.tensor.matmul(r1Tp[:], r1[0:1, 0:K], ones11[:],
                                 start=True, stop=True)
                r1Ts = small.tile([K, 1], F32)
                nc.vector.tensor_copy(r1Ts[:], r1Tp[:])
                for c in range(DC):
                    mp = psum.tile([128, 1], F32, tag="pre", name=f"mp{c}")
                    nc.tensor.matmul(mp[:], meansPt[:, c * 128:(c + 1) * 128],
                                     r1Ts[:], start=True, stop=True)
                    nc.vector.tensor_copy(mean1c[:, c:c + 1], mp[:])
                    cp = psum.tile([128, 1], F32, tag="sig", name=f"cp{c}")
                    nc.tensor.matmul(cp[:], covsPt[:, c * 128:(c + 1) * 128],
                                     r1Ts[:], start=True, stop=True)
                    nc.vector.tensor_copy(covs1c[:, c:c + 1], cp[:])
                # global agreement check: floored to the end of the
                # scheduler clock so the AR-dependent ops sit at the TAIL of
                # every engine stream (the AR itself runs concurrently with
                # the main loop).
                with tc.tile_wait_until(50):
                    lqg = small.tile([1, KP], F32)
                    nc.vector.scalar_tensor_tensor(lqg[:], qg[:], -0.5,
                                                   logp[:], ALU.mult, ALU.add)
                    mxg = small.tile([1, 1], F32)
                    nc.vector.tensor_reduce(mxg[:], lqg[:], AX.X, ALU.max)
                    nmxg = small.tile([1, 1], F32)
                    nc.vector.tensor_scalar(nmxg[:], mxg[:], -1.0, None,
                                            ALU.mult)
                    r1g = small.tile([1, KP], F32)
                    nc.vector.tensor_scalar(r1g[:], lqg[:], mxg[:], None,
                                            ALU.is_equal)
                    agv = small.tile([1, KP], F32)
                    nc.vector.tensor_tensor(agv[:], r1g[:], r1[:], ALU.mult)
                    agr = small.tile([1, 1], F32)
                    nc.vector.tensor_reduce(agr[:], agv[:], AX.X, ALU.add)
                    nc.vector.tensor_scalar(agr[:], agr[:], 1.0, None, ALU.min)
                    eg = small.tile([1, KP], F32)
                    nc.scalar.activation(eg[:], lqg[:], AF.Exp, bias=nmxg[:])
                    sg = small.tile([1, 1], F32)
                    nc.vector.tensor_reduce(sg[:], eg[:], AX.X, ALU.add)
                    rvg = small.tile([1, 1], F32)
                    nc.vector.reciprocal(rvg[:], sg[:])
                    dv = small.tile([1, 1], F32)
                    nc.vector.tensor_tensor(dv[:], rv1[:], rvg[:],
                                            ALU.subtract)
                    nc.vector.tensor_tensor(dv[:], dv[:], dv[:], ALU.mult)
                    ok2 = small.tile([1, 1], F32)
                    nc.vector.tensor_scalar(ok2[:], dv[:], 1.0e-6, None,
                                            ALU.is_lt)
                    flag = small.tile([1, 1], F32)
                    nc.vector.tensor_tensor(flag[:], agr[:], ok2[:], ALU.mult)
                    finv = small.tile([1, 1], F32, bufs=1)
                    nc.vector.reciprocal(finv[:], flag[:])
                if stage <= 4:
                    emit_dbg([(0, rvec[0:1, :]), (KP, rv1[0:1, :]),
                              (32, mean1c[0:1, :]), (40, covs1c[0:1, :]),
                              (48, flag[0:1, :])])

            stmp_cm.__exit__(None, None, None)

            # ---------- main loop ----------
            # ACT-stream ordering: Ln and Exp land in different walrus table
            # sets, so every Ln<->Exp boundary costs a ~1.3us ACT_TABLE_LOAD.
            # Sigma is staged PSUM->SBUF by the DVE so all 8 Ln's of a batch
            # tile run as one block; scheduler-clock floors (tile_wait_until)
            # are applied ONLY to ACT instructions, ordering the ACT stream
            # as [Ln x8][Exp x24 + universal][Gelu x8] per tile while leaving
            # PE/DVE/DMA free to overlap.
            if not done[0]:
                sigp_cm = tc.tile_pool(name="sigpool", bufs=1)
                sigpool = sigp_cm.__enter__()
                hmix_t, outs_t = {}, {}

                def stage_a(t):
                    bsl = slice(t * BT, (t + 1) * BT)
                    fb = 10.0 * (t + 1)
                    dm = [stash.tile([128, BT], F32R, tag=f"dm{j}",
                                     name=f"dm{t}_{j}") for j in range(DC)]
                    mc = [stash.tile([128, BT], BF16, tag=f"mc{j}",
                                     name=f"mc{t}_{j}") for j in range(DC)]
                    for j in range(DC):
                        nc.vector.scalar_tensor_tensor(
                            dm[j][:], msk[j][:, bsl], mean1c[:, j:j + 1],
                            xm[j][:, bsl], ALU.mult, ALU.add)
                        nc.vector.tensor_scalar(
                            mc[j][:], msk[j][:, bsl], covs1c[:, j:j + 1],
                            None, ALU.mult)

                    sigS = []
                    for hc in range(HC):
                        hsl = slice(hc * 128, (hc + 1) * 128)
                        sig = psum.tile([128, BT], F32, tag="sig",
                                        name=f"sig{t}_{hc}")
                        for j in range(DC):
                            nc.tensor.matmul(sig[:], w1s[j][:, hsl],
                                             mc[j][:], start=(j == 0),
                                             stop=(j == DC - 1))
                        ss = sigpool.tile([128, BT], F32, tag=f"ss{hc}",
                                          name=f"ss{t}_{hc}")
                        nc.vector.tensor_copy(ss[:], sig[:])
                        sigS.append(ss)
                    wst = []
                    for hc in range(HC):
                        hsl = slice(hc * 128, (hc + 1) * 128)
                        pre = psum.tile([128, BT], F32, tag="pre",
                                        name=f"pre{t}_{hc}")
                        for j in range(DC):
                            nc.tensor.matmul(pre[:], w1t[j][:, hsl],
                                             dm[j][:], start=(j == 0),
                                             stop=(j == DC - 1))
                        # wt_ holds mu = relu(pre + b1) for now; becomes
                        # w = mu/sqrt(sig) in place after rs is ready
                        wt_ = stash.tile([128, BT], F32, tag=f"wt{hc}",
                                         name=f"wt{t}_{hc}")
                        nc.vector.tensor_scalar(wt_[:], pre[:],
                                                b1t[:, hc:hc + 1], 0.0,
                                                ALU.add, ALU.max)
                        wst.append(wt_)

                    # Ln block (one table load); in place over staged sigma
                    with tc.tile_wait_until(fb):
                        for hc in range(HC):
                            nc.scalar.activation(sigS[hc][:], sigS[hc][:],
                                                 AF.Ln)

                    # Exp block (one table load); Square is set-universal
                    srst, zs = [], []
                    for hc in range(HC):
                        tl = sigS[hc]
                        with tc.tile_wait_until(fb + 1):
                            rs = trans.tile([128, BT], F32, tag="w2",
                                            name=f"rs{t}_{hc}", bufs=8)
                            nc.scalar.activation(rs[:], tl[:], AF.Exp,
                                                 scale=-0.5)
                            srw = stash.tile([128, BT], F32, tag=f"srw{hc}",
                                             name=f"srw{t}_{hc}")
                            nc.scalar.activation(srw[:], tl[:], AF.Exp,
                                                 scale=0.5, bias=lr128[:])
                        nc.vector.tensor_tensor(wst[hc][:], wst[hc][:],
                                                rs[:], ALU.mult)
                        w2 = trans.tile([128, BT], F32, tag="w2",
                                        name=f"w2{t}_{hc}", bufs=8)
                        with tc.tile_wait_until(fb + 1):
                            nc.scalar.activation(w2[:], wst[hc][:], AF.Square)
                        # z = t - w^2, in place over w2
                        nc.vector.tensor_tensor(w2[:], tl[:], w2[:],
                                                ALU.subtract)
                        srst.append(srw)
                        zs.append(w2)
                    # hm-Exp trailing block: same exp table, placed last so a
                    # straggler can't pull the gelu set in early
                    hmix = []
                    for hc in range(HC):
                        hm = stash.tile([128, BT], F32R, tag=f"hm{hc}",
                                        name=f"hm{t}_{hc}")
                        with tc.tile_wait_until(fb + 1.5):
                            nc.scalar.activation(hm[:], zs[hc][:], AF.Exp,
                                                 scale=0.5, bias=lcr128[:])
                        hmix.append(hm)

                    # Gelu block (one table load); in place over w
                    for hc in range(HC):
                        with tc.tile_wait_until(fb + 2):
                            nc.scalar.activation(wst[hc][:], wst[hc][:],
                                                 AF.Gelu)
                        nc.vector.tensor_tensor(wst[hc][:], srst[hc][:],
                                                wst[hc][:], ALU.mult)
                        nc.vector.tensor_tensor(hmix[hc][:], hmix[hc][:],
                                                wst[hc][:], ALU.add)
                    hmix_t[t] = hmix

                def stage_l(t):
                    bsl = slice(t * BT, (t + 1) * BT)
                    fb = 10.0 * (t + 1)
                    hmix = hmix_t[t]
                    h2s = []
                    for half in range(2):
                        h2ps = [psum.tile([128, BT], F32, tag="h2p", bufs=4,
                                          name=f"h2p{t}_{half}_{i}")
                                for i in range(4)]
                        for c0 in range(HC):
                            for i in range(4):
                                c1 = half * 4 + i
                                nc.tensor.matmul(
                                    h2ps[i][:],
                                    w2t[c0][:, c1 * 128:(c1 + 1) * 128],
                                    hmix[c0][:], start=(c0 == 0),
                                    stop=(c0 == HC - 1))
                        for i in range(4):
                            c1 = half * 4 + i
                            h2 = trans.tile([128, BT], F32R, tag="h2",
                                            name=f"h2{t}_{c1}")
                            with tc.tile_wait_until(fb + 2):
                                nc.scalar.activation(h2[:], h2ps[i][:],
                                                     AF.Prelu,
                                                     bias=b2t[:, c1:c1 + 1],
                                                     alpha=0.2)
                            h2s.append(h2)
                    outp = psum.tile([1, BT], F32, tag="sig",
                                     name=f"outp{t}")
                    for c1 in range(H1C):
                        nc.tensor.matmul(outp[:], w3t[:, c1:c1 + 1],
                                         h2s[c1][:], start=(c1 == 0),
                                         stop=(c1 == H1C - 1))
                    with tc.tile_wait_until(51):
                        outs = small.tile([1, BT], F32, tag="outs",
                                          name=f"outs{t}")
                        nc.vector.tensor_scalar(outs[:], outp[:], b3t[:],
                                                finv[:], ALU.add, ALU.mult)
                        nc.sync.dma_start(out[0:1, bsl], outs[:])

                # A(0) A(1) L(0) L(1): the t=1 layer-1 matmuls keep the PE
                # busy during t=0's gelu phase, and L2(0) fills t=1's.
                stage_a(0)
                stage_l(0)
                stage_a(1)
                stage_l(1)
                sigp_cm.__exit__(None, None, None)

    nc.compile()
    return nc


def _prep_inputs(x, means, covs, weights_mix, gamma, W1, b1, W2, b2, W3, b3):
    x = np.asarray(x, np.float32)
    means = np.ascontiguousarray(np.asarray(means, np.float32))
    covs = np.ascontiguousarray(np.asarray(covs, np.float32))
    W1 = np.asarray(W1, np.float32)
    W2 = np.asarray(W2, np.float32)
    W3 = np.asarray(W3, np.float32)
    wmixv = np.full((1, KP), NEG_BIG, np.float32)
    wmixv[0, :K] = np.asarray(weights_mix, np.float32)
    common = {
        "meansP": means,
        "covsP": covs,
        "meansT": np.ascontiguousarray(means.T),
        "covsT": np.ascontiguousarray(covs.T),
        "gamma128": np.full((128, 1), np.float32(gamma), np.float32),
        "wmix": wmixv,
        "W1T": np.ascontiguousarray(W1.T),
        "W1sqT": np.ascontiguousarray((W1 * W1).T).astype(ml_dtypes.bfloat16),
        "W2T": np.ascontiguousarray(W2.T),
        "w3c": np.ascontiguousarray(W3.reshape(H1C, 128).T),
        "b1c": np.ascontiguousarray(np.asarray(b1, np.float32).reshape(HC, 128).T),
        "b2c": np.ascontiguousarray(np.asarray(b2, np.float32).reshape(H1C, 128).T),
        "b3": np.asarray(b3, np.float32).reshape(1, 1),
    }
    in_maps = []
    for i in range(N_CORES):
        shard = x[i * BS:(i + 1) * BS]
        m = dict(common)
        m["xT"] = np.ascontiguousarray(shard.T)
        in_maps.append(m)
    return in_maps


def kernel(**inputs):
    if "nc" not in _CACHE:
        _CACHE["nc"] = _build()
    nc = _CACHE["nc"]
    in_maps = _prep_inputs(**inputs)
    res = run_bass_kernel_spmd(nc, in_maps, list(range(N_CORES)))
    parts = [res.results[i]["out"][0] for i in range(N_CORES)]
    return np.concatenate(parts).reshape(B, 1).astype(np.float32)
